# revision 2
# baseline (speedup 1.0000x reference)
"""AttentionGRUCell feature-major bf16 kernel for 8 Trainium2 NeuronCores.

Data-parallel over batch (2048 rows/core).  All activations live in
feature-major layout [K, batch]: the host pre-transposes x/h/a into one
stacked slab actT = [xT; hT; aT] ([4096, 2048] per core) and pre-swizzles
it into DMA-friendly [block, p, kb, n] tiles, so every matmul operand is
a straight contiguous DMA: no DMA-XBAR transposes and no DRAM spills.

Matmuls put the weight k-block [128, 128] stationary (lhsT) and stream
512 batch columns (rhs), producing pre-activations directly in
[feature, batch] layout.  Intermediates (z, rh, s) therefore stay in
SBUF in exactly the layout the next gate's matmul needs.

Weights are stacked per gate along K ([Wg; Ug; Cg] -> [4096, 1024]),
host-swizzled into 4 column chunks [128, 32kb, 256m], and streamed per
(block, gate) on the sync queue with a 3-buffer rolling pool.  Act slabs
stream on the scalar queue; output stores go on gpsimd (SWDGE).

Per batch-block of 512 columns the four gates run back-to-back:
  z: 8x32 MM -> sigmoid -> z16 (SBUF)
  r: 8x32 MM -> sigmoid -> rh16 = r*h (SBUF)
  s: 8x32 MM (x, a, then rh last) -> tanh -> s = h + z*(s~-h)
     -> s16 (SBUF) + store
  t: 8x32 MM (x, a, then s16 last) -> relu -> store
Outputs are written feature-major in bf16; the host transposes back and
upcasts to f32.
"""

import sys

if "/opt/trn_rl_repo" not in sys.path:
    sys.path.insert(0, "/opt/trn_rl_repo")

import numpy as np

BATCH = 16384
EMB = 1024
HID = 1024
COMB = 2048
KTOT = EMB + HID + COMB          # 4096 stacked K: [x | mid | a]
N_CORES = 8
B_L = BATCH // N_CORES           # 2048 batch columns per core
P = 128
NW = 512                         # batch columns per block
NBLK = B_L // NW                 # 4
NKB = KTOT // P                  # 32 k-blocks: x 0-7, mid 8-15, a 16-31
CM = 256                         # weight chunk width (output features)
NCHUNK = HID // CM               # 4 chunks per gate


def _build_nc(tweak="", with_bias=False):
    import concourse.mybir as mybir
    from contextlib import ExitStack
    from concourse import bacc
    from concourse.tile import TileContext

    dt = mybir.dt
    AF = mybir.ActivationFunctionType

    nc = bacc.Bacc("TRN2", target_bir_lowering=False, debug=False,
                   num_devices=N_CORES)

    acts_d = nc.declare_dram_parameter("acts", [NBLK * P, NKB * NW],
                                       dt.bfloat16, isOutput=False)
    wd = {g: nc.declare_dram_parameter(f"w{g}", [NCHUNK * P, NKB * CM],
                                       dt.bfloat16, isOutput=False)
          for g in "zrst"}
    bias_d = None
    if with_bias:
        bias_d = nc.declare_dram_parameter("bias", [P, 32], dt.float32,
                                           isOutput=False)
    s_out = nc.declare_dram_parameter("s", [HID, B_L], dt.bfloat16,
                                      isOutput=True)
    t_out = nc.declare_dram_parameter("t", [HID, B_L], dt.bfloat16,
                                      isOutput=True)

    wbufs = 3 if "w3" in tweak else 4
    psbufs = 5 if "ps5" in tweak else 6

    with TileContext(nc) as tc, ExitStack() as top:
        wp = top.enter_context(tc.tile_pool(name="wp", bufs=wbufs))
        sp = top.enter_context(tc.tile_pool(name="sp", bufs=2))
        mp = top.enter_context(tc.tile_pool(name="mp", bufs=2))
        ep = top.enter_context(tc.tile_pool(name="ep", bufs=4))
        ps = top.enter_context(tc.tile_pool(name="ps", bufs=psbufs,
                                            space="PSUM"))
        bp = top.enter_context(tc.tile_pool(name="bp", bufs=1))

        bias_t = None
        if with_bias:
            bias_t = bp.tile([P, 32], dt.float32, tag="bias")
            nc.sync.dma_start(bias_t[:], bias_d[:])

        def bias_ap(gi, m):
            if bias_t is None:
                return 0.0
            return bias_t[:, gi * 8 + m:gi * 8 + m + 1]

        # all act slab loads up front on the scalar queue; bufs=2 throttles.
        # quarter-granularity (8 kb each) so the first matmuls start early.
        slabs = []
        for b in range(NBLK):
            st_ = sp.tile([P, NKB, NW], dt.bfloat16, tag="slab")
            r0 = b * P
            for q in range(4):
                nc.scalar.dma_start(
                    st_[:, 8 * q:8 * (q + 1), :],
                    acts_d[r0:r0 + P, 8 * q * NW:8 * (q + 1) * NW])
            slabs.append(st_)

        def gate_weights(g):
            tiles = []
            for c in range(NCHUNK):
                wt = wp.tile([P, NKB, CM], dt.bfloat16, tag="wch")
                nc.sync.dma_start(wt[:, 0:16, :],
                                  wd[g][c * P:(c + 1) * P, 0:16 * CM])
                nc.sync.dma_start(wt[:, 16:32, :],
                                  wd[g][c * P:(c + 1) * P, 16 * CM:32 * CM])
                tiles.append(wt)
            return tiles

        KB_X = list(range(0, 8))
        KB_MID = list(range(8, 16))
        KB_A = list(range(16, 32))

        for b in range(NBLK):
            slab = slabs[b]
            z16 = mp.tile([P, 8, NW], dt.bfloat16, tag="z16")
            rh16 = mp.tile([P, 8, NW], dt.bfloat16, tag="rh16")
            s16 = mp.tile([P, 8, NW], dt.bfloat16, tag="s16")

            for gi, g in enumerate("zrst"):
                wts = gate_weights(g)
                if g in "zr":
                    kb_order = KB_X + KB_MID + KB_A

                    def rhs_of(kb, slab=slab):
                        return slab[:, kb, :]
                else:
                    mid = rh16 if g == "s" else s16
                    kb_order = KB_X + KB_A + KB_MID

                    def rhs_of(kb, slab=slab, mid=mid):
                        if 8 <= kb < 16:
                            return mid[:, kb - 8, :]
                        return slab[:, kb, :]

                for m in range(8):
                    wt = wts[m // 2]
                    mo = (m % 2) * P
                    pp = ps.tile([P, NW], dt.float32, tag="ps")
                    for i, kb in enumerate(kb_order):
                        nc.tensor.matmul(pp[:], wt[:, kb, mo:mo + P],
                                         rhs_of(kb),
                                         start=(i == 0), stop=(i == NKB - 1))
                    h_m = slab[:, 8 + m, :]
                    if g == "z":
                        nc.scalar.activation(z16[:, m, :], pp[:], AF.Sigmoid,
                                             bias=bias_ap(0, m))
                    elif g == "r":
                        rm = ep.tile([P, NW], dt.bfloat16, tag="rm")
                        nc.scalar.activation(rm[:], pp[:], AF.Sigmoid,
                                             bias=bias_ap(1, m))
                        nc.vector.tensor_mul(rh16[:, m, :], rm[:], h_m)
                    elif g == "s":
                        stm = ep.tile([P, NW], dt.float32, tag="stm")
                        nc.scalar.activation(stm[:], pp[:], AF.Tanh,
                                             bias=bias_ap(2, m))
                        nc.vector.tensor_sub(stm[:], stm[:], h_m)
                        nc.vector.tensor_mul(stm[:], z16[:, m, :], stm[:])
                        nc.vector.tensor_add(s16[:, m, :], h_m, stm[:])
                        nc.gpsimd.dma_start(
                            s_out[m * P:(m + 1) * P, b * NW:(b + 1) * NW],
                            s16[:, m, :])
                    else:
                        tm = ep.tile([P, NW], dt.bfloat16, tag="tm")
                        nc.scalar.activation(tm[:], pp[:], AF.Relu,
                                             bias=bias_ap(3, m))
                        nc.gpsimd.dma_start(
                            t_out[m * P:(m + 1) * P, b * NW:(b + 1) * NW],
                            tm[:])

    nc.compile()
    return nc


_CACHE = {}


def _get_exec(tweak="", with_bias=False):
    key = (tweak, with_bias)
    if key in _CACHE:
        return _CACHE[key]

    import jax
    import concourse.mybir as mybir
    from concourse import bass2jax
    from jax.sharding import Mesh, PartitionSpec
    from jax.experimental.shard_map import shard_map

    bass2jax.install_neuronx_cc_hook()
    nc = _build_nc(tweak, with_bias)

    partition_name = (nc.partition_id_tensor.name
                      if nc.partition_id_tensor else None)
    in_names = []
    out_names = []
    out_avals = []
    zero_outs = []
    for alloc in nc.m.functions[0].allocations:
        if not isinstance(alloc, mybir.MemoryLocationSet):
            continue
        name = alloc.memorylocations[0].name
        if alloc.kind == "ExternalInput":
            if name != partition_name:
                in_names.append(name)
        elif alloc.kind == "ExternalOutput":
            out_names.append(name)
            shape = tuple(alloc.tensor_shape)
            dtype = mybir.dt.np(alloc.dtype)
            out_avals.append(jax.core.ShapedArray(shape, dtype))
            zero_outs.append(np.zeros(shape, dtype))
    n_params = len(in_names)
    all_in_names = in_names + out_names
    if partition_name is not None:
        all_in_names = all_in_names + [partition_name]

    def _body(*args):
        operands = list(args)
        if partition_name is not None:
            operands.append(bass2jax.partition_id_tensor())
        outs = bass2jax._bass_exec_p.bind(
            *operands,
            out_avals=tuple(out_avals),
            in_names=tuple(all_in_names),
            out_names=tuple(out_names),
            lowering_input_output_aliases=(),
            sim_require_finite=True,
            sim_require_nnan=True,
            nc=nc,
        )
        return tuple(outs)

    devices = jax.devices()[:N_CORES]
    mesh = Mesh(np.asarray(devices), ("core",))
    n_outs = len(out_names)
    sharded = jax.jit(
        shard_map(
            _body, mesh=mesh,
            in_specs=(PartitionSpec("core"),) * (n_params + n_outs),
            out_specs=(PartitionSpec("core"),) * n_outs,
            check_rep=False,
        ),
        keep_unused=True,
    )
    entry = {
        "nc": nc,
        "sharded": sharded,
        "in_names": in_names,
        "out_names": out_names,
        "zero_outs": zero_outs,
        "mesh": mesh,
    }
    _CACHE[key] = entry
    return entry


def _swizzle_acts(x, h, a):
    """[16384, {1024,1024,2048}] f32 -> [8*512, 16384] bf16 slab layout.

    acts[c*512 + b*128 + p, kb*512 + j] = actT_c[kb*128 + p, b*512 + j]
    where actT_c = [x_c.T; h_c.T; a_c.T].
    """
    import ml_dtypes
    bf16 = ml_dtypes.bfloat16
    full = np.concatenate(
        [np.asarray(x, np.float32), np.asarray(h, np.float32),
         np.asarray(a, np.float32)], axis=1).astype(bf16)   # [B, K]
    # full[c*2048 + b*512 + j, kb*128 + p] -> out[c][b][p][kb][j]
    out = full.reshape(N_CORES, NBLK, NW, NKB, P).transpose(0, 1, 4, 3, 2)
    return np.ascontiguousarray(out.reshape(N_CORES * NBLK * P, NKB * NW))


def _swizzle_gate_weights(mats):
    """[Wg, Ug, Cg] f32 -> [4*128, 32*256] bf16 chunk layout (one core).

    w[c*128 + p, kb*256 + mc] = G[kb*128 + p, c*256 + mc]
    """
    import ml_dtypes
    bf16 = ml_dtypes.bfloat16
    G = np.concatenate([np.asarray(m, np.float32) for m in mats],
                       axis=0).astype(bf16)                  # [4096, 1024]
    out = G.reshape(NKB, P, NCHUNK, CM).transpose(2, 1, 0, 3)
    return np.ascontiguousarray(out.reshape(NCHUNK * P, NKB * CM))


def _prepare_in_arrays(entry, inputs, bias_rows=None):
    acts = _swizzle_acts(inputs["in_word"], inputs["last_hid_state"],
                         inputs["attended_state"])
    gate_mats = {
        "wz": [inputs["Wz"], inputs["Uz"], inputs["Cz"]],
        "wr": [inputs["Wr"], inputs["Ur"], inputs["Cr"]],
        "ws": [inputs["W"], inputs["U"], inputs["C"]],
        "wt": [inputs["Vo"], inputs["Uo"], inputs["Co"]],
    }
    arrs = []
    for name in entry["in_names"]:
        if name == "acts":
            arrs.append(acts)
        elif name == "bias":
            cols = []
            for g in "zrst":
                cols.append(np.asarray(bias_rows[g], np.float32)
                            .reshape(8, P).T)
            bt = np.concatenate(cols, axis=1)               # [128, 32]
            arrs.append(np.ascontiguousarray(np.tile(bt, (N_CORES, 1))))
        else:
            w = _swizzle_gate_weights(gate_mats[name])
            arrs.append(np.ascontiguousarray(np.tile(w, (N_CORES, 1))))
    return arrs


def _unswizzle_out(dev):
    """[8*1024, 2048] bf16 feature-major -> [16384, 1024] f32."""
    arr = np.asarray(dev)
    return (arr.reshape(N_CORES, HID, B_L).transpose(0, 2, 1)
            .reshape(BATCH, HID).astype(np.float32))


def kernel(in_word, last_hid_state, attended_state,
           W, bw, Wz, bwz, Wr, bwr,
           U, bu, Uz, buz, Ur, bur,
           C, bc, Cz, bcz, Cr, bcr,
           Uo, buo, Vo, bvo, Co, bco):
    inputs = dict(in_word=np.asarray(in_word),
                  last_hid_state=np.asarray(last_hid_state),
                  attended_state=np.asarray(attended_state),
                  W=W, Wz=Wz, Wr=Wr, U=U, Uz=Uz, Ur=Ur,
                  C=C, Cz=Cz, Cr=Cr, Uo=Uo, Vo=Vo, Co=Co)
    bias_rows = {
        "z": np.asarray(bwz) + np.asarray(buz) + np.asarray(bcz),
        "r": np.asarray(bwr) + np.asarray(bur) + np.asarray(bcr),
        "s": np.asarray(bw) + np.asarray(bu) + np.asarray(bc),
        "t": np.asarray(buo) + np.asarray(bvo) + np.asarray(bco),
    }
    with_bias = bool(any(np.any(np.asarray(v) != 0)
                         for v in bias_rows.values()))

    entry = _get_exec(with_bias=with_bias)
    arrs = _prepare_in_arrays(entry, inputs, bias_rows)
    zeros = [np.zeros((N_CORES * z.shape[0], *z.shape[1:]), z.dtype)
             for z in entry["zero_outs"]]
    outs = entry["sharded"](*arrs, *zeros)
    res = {name: outs[i] for i, name in enumerate(entry["out_names"])}
    return (_unswizzle_out(res["s"]), _unswizzle_out(res["t"]))


# revision 3
# speedup vs baseline: 1.0335x; 1.0335x over previous
"""AttentionGRUCell feature-major bf16 kernel for 8 Trainium2 NeuronCores.

Same design as kernel_v2 (feature-major, host-swizzled operands, weights
stationary, batch streaming, intermediates SBUF-resident) with two deltas:
  - all four gates' weights ship as ONE DRAM input ("wall") to cut per-call
    argument count;
  - cold-start balancing: block 0's 4th slab quarter loads on the sync ring
    after gate z's first weight chunk, so the first psum group completes at
    ~8.5 us instead of ~11 us.
"""

import sys

if "/opt/trn_rl_repo" not in sys.path:
    sys.path.insert(0, "/opt/trn_rl_repo")

import numpy as np

BATCH = 16384
EMB = 1024
HID = 1024
COMB = 2048
KTOT = EMB + HID + COMB          # 4096 stacked K: [x | mid | a]
N_CORES = 8
B_L = BATCH // N_CORES           # 2048 batch columns per core
P = 128
NW = 512                         # batch columns per block
NBLK = B_L // NW                 # 4
NKB = KTOT // P                  # 32 k-blocks: x 0-7, mid 8-15, a 16-31
CM = 256                         # weight chunk width (output features)
NCHUNK = HID // CM               # 4 chunks per gate


def _build_nc(tweak="", with_bias=False):
    import concourse.mybir as mybir
    from contextlib import ExitStack
    from concourse import bacc
    from concourse.tile import TileContext

    dt = mybir.dt
    AF = mybir.ActivationFunctionType

    nc = bacc.Bacc("TRN2", target_bir_lowering=False, debug=False,
                   num_devices=N_CORES)

    acts_d = nc.declare_dram_parameter("acts", [NBLK * P, NKB * NW],
                                       dt.bfloat16, isOutput=False)
    wall_d = nc.declare_dram_parameter("wall", [4 * NCHUNK * P, NKB * CM],
                                       dt.bfloat16, isOutput=False)
    bias_d = None
    if with_bias:
        bias_d = nc.declare_dram_parameter("bias", [P, 32], dt.float32,
                                           isOutput=False)
    s_out = nc.declare_dram_parameter("s", [HID, B_L], dt.bfloat16,
                                      isOutput=True)
    t_out = nc.declare_dram_parameter("t", [HID, B_L], dt.bfloat16,
                                      isOutput=True)

    wbufs = 3 if "w3" in tweak else 4
    psbufs = 5 if "ps5" in tweak else 6

    with TileContext(nc) as tc, ExitStack() as top:
        wp = top.enter_context(tc.tile_pool(name="wp", bufs=wbufs))
        sp = top.enter_context(tc.tile_pool(name="sp", bufs=2))
        mp = top.enter_context(tc.tile_pool(name="mp", bufs=2))
        ep = top.enter_context(tc.tile_pool(name="ep", bufs=4))
        ps = top.enter_context(tc.tile_pool(name="ps", bufs=psbufs,
                                            space="PSUM"))
        bp = top.enter_context(tc.tile_pool(name="bp", bufs=1))

        bias_t = None
        if with_bias:
            bias_t = bp.tile([P, 32], dt.float32, tag="bias")
            nc.sync.dma_start(bias_t[:], bias_d[:])

        def bias_ap(gi, m):
            if bias_t is None:
                return 0.0
            return bias_t[:, gi * 8 + m:gi * 8 + m + 1]

        # act slab loads at quarter granularity.  Block 0 is the cold-start
        # critical path: its last quarter is deferred to the sync ring and
        # issued right after gate z's first weight chunk (see below), so the
        # scalar ring only carries 3 quarters before the first group closes.
        slabs = []
        deferred_q3 = []
        for b in range(NBLK):
            st_ = sp.tile([P, NKB, NW], dt.bfloat16, tag="slab")
            r0 = b * P
            for q in range(4):
                if b == 0 and q == 3:
                    deferred_q3.append((st_, r0))
                    continue
                nc.scalar.dma_start(
                    st_[:, 8 * q:8 * (q + 1), :],
                    acts_d[r0:r0 + P, 8 * q * NW:8 * (q + 1) * NW])
            slabs.append(st_)

        def gate_weights(gi):
            tiles = []
            for c in range(NCHUNK):
                wt = wp.tile([P, NKB, CM], dt.bfloat16, tag="wch")
                r0 = (gi * NCHUNK + c) * P
                nc.sync.dma_start(wt[:, 0:16, :],
                                  wall_d[r0:r0 + P, 0:16 * CM])
                nc.sync.dma_start(wt[:, 16:32, :],
                                  wall_d[r0:r0 + P, 16 * CM:32 * CM])
                if deferred_q3 and gi == 0 and c == 0:
                    st0, sr0 = deferred_q3.pop()
                    nc.sync.dma_start(
                        st0[:, 24:32, :],
                        acts_d[sr0:sr0 + P, 24 * NW:32 * NW])
                tiles.append(wt)
            return tiles

        KB_X = list(range(0, 8))
        KB_MID = list(range(8, 16))
        KB_A = list(range(16, 32))

        for b in range(NBLK):
            slab = slabs[b]
            z16 = mp.tile([P, 8, NW], dt.bfloat16, tag="z16")
            rh16 = mp.tile([P, 8, NW], dt.bfloat16, tag="rh16")
            s16 = mp.tile([P, 8, NW], dt.bfloat16, tag="s16")

            for gi, g in enumerate("zrst"):
                wts = gate_weights(gi)
                if g in "zr":
                    kb_order = KB_X + KB_MID + KB_A

                    def rhs_of(kb, slab=slab):
                        return slab[:, kb, :]
                else:
                    mid = rh16 if g == "s" else s16
                    kb_order = KB_X + KB_A + KB_MID

                    def rhs_of(kb, slab=slab, mid=mid):
                        if 8 <= kb < 16:
                            return mid[:, kb - 8, :]
                        return slab[:, kb, :]

                for m in range(8):
                    wt = wts[m // 2]
                    mo = (m % 2) * P
                    pp = ps.tile([P, NW], dt.float32, tag="ps")
                    for i, kb in enumerate(kb_order):
                        nc.tensor.matmul(pp[:], wt[:, kb, mo:mo + P],
                                         rhs_of(kb),
                                         start=(i == 0), stop=(i == NKB - 1))
                    h_m = slab[:, 8 + m, :]
                    if g == "z":
                        nc.scalar.activation(z16[:, m, :], pp[:], AF.Sigmoid,
                                             bias=bias_ap(0, m))
                    elif g == "r":
                        rm = ep.tile([P, NW], dt.bfloat16, tag="rm")
                        nc.scalar.activation(rm[:], pp[:], AF.Sigmoid,
                                             bias=bias_ap(1, m))
                        nc.vector.tensor_mul(rh16[:, m, :], rm[:], h_m)
                    elif g == "s":
                        stm = ep.tile([P, NW], dt.float32, tag="stm")
                        nc.scalar.activation(stm[:], pp[:], AF.Tanh,
                                             bias=bias_ap(2, m))
                        nc.vector.tensor_sub(stm[:], stm[:], h_m)
                        nc.vector.tensor_mul(stm[:], z16[:, m, :], stm[:])
                        nc.vector.tensor_add(s16[:, m, :], h_m, stm[:])
                        nc.gpsimd.dma_start(
                            s_out[m * P:(m + 1) * P, b * NW:(b + 1) * NW],
                            s16[:, m, :])
                    else:
                        tm = ep.tile([P, NW], dt.bfloat16, tag="tm")
                        nc.scalar.activation(tm[:], pp[:], AF.Relu,
                                             bias=bias_ap(3, m))
                        tq = nc.sync if b == NBLK - 1 else nc.gpsimd
                        tq.dma_start(
                            t_out[m * P:(m + 1) * P, b * NW:(b + 1) * NW],
                            tm[:])

    nc.compile()
    return nc


_CACHE = {}


def _get_exec(tweak="", with_bias=False):
    key = (tweak, with_bias)
    if key in _CACHE:
        return _CACHE[key]

    import jax
    import concourse.mybir as mybir
    from concourse import bass2jax
    from jax.sharding import Mesh, PartitionSpec
    from jax.experimental.shard_map import shard_map

    bass2jax.install_neuronx_cc_hook()
    nc = _build_nc(tweak, with_bias)

    partition_name = (nc.partition_id_tensor.name
                      if nc.partition_id_tensor else None)
    in_names = []
    out_names = []
    out_avals = []
    zero_outs = []
    for alloc in nc.m.functions[0].allocations:
        if not isinstance(alloc, mybir.MemoryLocationSet):
            continue
        name = alloc.memorylocations[0].name
        if alloc.kind == "ExternalInput":
            if name != partition_name:
                in_names.append(name)
        elif alloc.kind == "ExternalOutput":
            out_names.append(name)
            shape = tuple(alloc.tensor_shape)
            dtype = mybir.dt.np(alloc.dtype)
            out_avals.append(jax.core.ShapedArray(shape, dtype))
            zero_outs.append(np.zeros(shape, dtype))
    n_params = len(in_names)
    all_in_names = in_names + out_names
    if partition_name is not None:
        all_in_names = all_in_names + [partition_name]

    def _body(*args):
        operands = list(args)
        if partition_name is not None:
            operands.append(bass2jax.partition_id_tensor())
        outs = bass2jax._bass_exec_p.bind(
            *operands,
            out_avals=tuple(out_avals),
            in_names=tuple(all_in_names),
            out_names=tuple(out_names),
            lowering_input_output_aliases=(),
            sim_require_finite=True,
            sim_require_nnan=True,
            nc=nc,
        )
        return tuple(outs)

    devices = jax.devices()[:N_CORES]
    mesh = Mesh(np.asarray(devices), ("core",))
    n_outs = len(out_names)
    sharded = jax.jit(
        shard_map(
            _body, mesh=mesh,
            in_specs=(PartitionSpec("core"),) * (n_params + n_outs),
            out_specs=(PartitionSpec("core"),) * n_outs,
            check_rep=False,
        ),
        keep_unused=True,
    )
    entry = {
        "nc": nc,
        "sharded": sharded,
        "in_names": in_names,
        "out_names": out_names,
        "zero_outs": zero_outs,
        "mesh": mesh,
    }
    _CACHE[key] = entry
    return entry


def _swizzle_acts(x, h, a):
    """[16384, {1024,1024,2048}] f32 -> [8*512, 16384] bf16 slab layout."""
    import ml_dtypes
    bf16 = ml_dtypes.bfloat16
    full = np.concatenate(
        [np.asarray(x, np.float32), np.asarray(h, np.float32),
         np.asarray(a, np.float32)], axis=1).astype(bf16)   # [B, K]
    out = full.reshape(N_CORES, NBLK, NW, NKB, P).transpose(0, 1, 4, 3, 2)
    return np.ascontiguousarray(out.reshape(N_CORES * NBLK * P, NKB * NW))


def _swizzle_gate_weights(mats):
    """[Wg, Ug, Cg] f32 -> [4*128, 32*256] bf16 chunk layout (one core)."""
    import ml_dtypes
    bf16 = ml_dtypes.bfloat16
    G = np.concatenate([np.asarray(m, np.float32) for m in mats],
                       axis=0).astype(bf16)                  # [4096, 1024]
    out = G.reshape(NKB, P, NCHUNK, CM).transpose(2, 1, 0, 3)
    return np.ascontiguousarray(out.reshape(NCHUNK * P, NKB * CM))


def _prepare_in_arrays(entry, inputs, bias_rows=None):
    acts = _swizzle_acts(inputs["in_word"], inputs["last_hid_state"],
                         inputs["attended_state"])
    gate_mats = [
        [inputs["Wz"], inputs["Uz"], inputs["Cz"]],
        [inputs["Wr"], inputs["Ur"], inputs["Cr"]],
        [inputs["W"], inputs["U"], inputs["C"]],
        [inputs["Vo"], inputs["Uo"], inputs["Co"]],
    ]
    arrs = []
    for name in entry["in_names"]:
        if name == "acts":
            arrs.append(acts)
        elif name == "bias":
            cols = []
            for g in range(4):
                cols.append(np.asarray(bias_rows["zrst"[g]], np.float32)
                            .reshape(8, P).T)
            bt = np.concatenate(cols, axis=1)               # [128, 32]
            arrs.append(np.ascontiguousarray(np.tile(bt, (N_CORES, 1))))
        else:   # wall
            wall = np.concatenate(
                [_swizzle_gate_weights(mats) for mats in gate_mats], axis=0)
            arrs.append(np.ascontiguousarray(np.tile(wall, (N_CORES, 1))))
    return arrs


def _unswizzle_out(dev):
    """[8*1024, 2048] bf16 feature-major -> [16384, 1024] f32."""
    arr = np.asarray(dev)
    return (arr.reshape(N_CORES, HID, B_L).transpose(0, 2, 1)
            .reshape(BATCH, HID).astype(np.float32))


def kernel(in_word, last_hid_state, attended_state,
           W, bw, Wz, bwz, Wr, bwr,
           U, bu, Uz, buz, Ur, bur,
           C, bc, Cz, bcz, Cr, bcr,
           Uo, buo, Vo, bvo, Co, bco):
    inputs = dict(in_word=np.asarray(in_word),
                  last_hid_state=np.asarray(last_hid_state),
                  attended_state=np.asarray(attended_state),
                  W=W, Wz=Wz, Wr=Wr, U=U, Uz=Uz, Ur=Ur,
                  C=C, Cz=Cz, Cr=Cr, Uo=Uo, Vo=Vo, Co=Co)
    bias_rows = {
        "z": np.asarray(bwz) + np.asarray(buz) + np.asarray(bcz),
        "r": np.asarray(bwr) + np.asarray(bur) + np.asarray(bcr),
        "s": np.asarray(bw) + np.asarray(bu) + np.asarray(bc),
        "t": np.asarray(buo) + np.asarray(bvo) + np.asarray(bco),
    }
    with_bias = bool(any(np.any(np.asarray(v) != 0)
                         for v in bias_rows.values()))

    entry = _get_exec(with_bias=with_bias)
    arrs = _prepare_in_arrays(entry, inputs, bias_rows)
    zeros = [np.zeros((N_CORES * z.shape[0], *z.shape[1:]), z.dtype)
             for z in entry["zero_outs"]]
    outs = entry["sharded"](*arrs, *zeros)
    res = {name: outs[i] for i, name in enumerate(entry["out_names"])}
    return (_unswizzle_out(res["s"]), _unswizzle_out(res["t"]))
